# revision 1
# baseline (speedup 1.0000x reference)
"""EdgeConv GNN (4 layers) on 8 Trainium2 NeuronCores.

Algebraic restructure: with y = x @ theta_w.T and
v = x @ (phi_w - theta_w).T + (phi_b + theta_b),
    msg_e = theta(x[src]-x[dst]) + theta_b + phi(x[dst]) + phi_b
          = y[src] + v[dst]
and since v[dst] is constant within a dst segment:
    out = relu(v + segment_max(y[src], dst))
(nodes with no in-edges come out of segment_max at -1e30 -> relu -> 0,
matching the reference's where(isneginf, 0) + relu).

Distribution: nodes sharded by dst across 8 cores (graph parallel).
Each layer: per-core matmuls produce its y-shard (cast to bf16) ->
AllGather the full bf16 y table to every core's DRAM -> SWDGE
dma_gather of 256B bf16 y rows by src in dst-sorted slot order ->
strided reduce_max per 128-node block (bf16), + v (f32) -> relu.

Perf notes (measured on HW):
- SWDGE dma_gather is descriptor-GENERATION-bound on the Q7: ~8.6ns/idx
  regardless of dtype (f32/bf16), call size (512/1024), queue count, or
  single_packet. GpSimd is ~89% busy; it is the kernel's critical path.
- num_idxs > 1024 per call faults the gather ucode (scratch cap).
- bf16 halves AllGather/CC time (~50us/layer) and DMA bytes; rel err
  ~3.2e-3 (tolerance 2e-2).
- 4-layer total ~4.06ms; gather gen floor for this slot layout
  (99328 slots/layer) is ~3.4ms of Q7 time.

dma_gather indices are int16 (<= 32767) so the 50176-row table is
addressed through two windows: A = rows [0, 25088) (= src cores 0-3)
and B = rows [18816, 50176) (src cores 3-7), each edge assigned to one
window (core-3 srcs balance the two).  Per-core slot structure must be
identical across cores (single SPMD instruction stream), so block
degree caps K are maxima across all 8 cores.
"""

import numpy as np

N = 50000
NCORES = 8
NPC = 6250            # real nodes per core
NPCP = 6272           # padded nodes per core (49 * 128)
F = 128
NL = 4
NB = NPCP // 128      # 49 blocks per core
NTAB = NCORES * NPCP  # 50176 table rows
BASE_B = 3 * NPCP     # 18816: window B base row
N_PHANTOM = NPCP - NPC
GMAX = 40             # max chunks per gather group (per window)
NEG = -1.0e30
SKIP = set()          # debug: subset of {"gather", "reduce", "ag", "mm"}

_cache = {}


# ----------------------------------------------------------------------------
# host-side graph preprocessing
# ----------------------------------------------------------------------------

def _prep_graph(src, dst):
    src = np.asarray(src).astype(np.int64)
    dst = np.asarray(dst).astype(np.int64)
    E = len(src)
    s_core = src // NPC
    d_core = dst // NPC

    fixedA = s_core <= 2
    flex = s_core == 3
    dA0 = np.bincount(dst[fixedA], minlength=N)
    dB0 = np.bincount(dst[s_core >= 4], minlength=N)
    dfx = np.bincount(dst[flex], minlength=N)
    kAf = np.clip((dB0 - dA0 + dfx + 1) // 2, 0, dfx)
    dA = dA0 + kAf
    dB = dB0 + (dfx - kAf)

    # edge side: fixed by src core; flex edges ranked within their dst group
    sideA = fixedA.copy()
    fe = np.flatnonzero(flex)
    fe = fe[np.argsort(dst[fe], kind="stable")]
    dsf = dst[fe]
    # rank within consecutive same-dst run
    starts = np.r_[0, np.flatnonzero(np.diff(dsf)) + 1]
    runlen = np.diff(np.r_[starts, len(dsf)])
    rank = np.arange(len(dsf)) - np.repeat(starts, runlen)
    sideA[fe[rank < kAf[dsf]]] = True

    # per-core node order: phantoms at positions 0..21, real sorted by
    # (dB, dA) — dB primary: dB is the wider-spread coordinate (window B
    # covers more src cores), so packing it tightly minimizes the summed
    # block caps (758 vs 776 chunks for dA-primary on this graph).
    pos = np.empty(N, np.int64)
    for c in range(NCORES):
        ids = np.arange(c * NPC, (c + 1) * NPC)
        order = np.lexsort((dA[ids], dB[ids]))
        pos[ids[order]] = N_PHANTOM + np.arange(NPC)
    sig = (np.arange(N) // NPC) * NPCP + pos  # orig node -> table row
    blk = pos // 128
    lane = pos % 128

    # global (cross-core) block degree caps
    KA = np.zeros(NB, np.int64)
    KB = np.zeros(NB, np.int64)
    np.maximum.at(KA, blk, dA)
    np.maximum.at(KB, blk, dB)
    cbA = np.r_[0, np.cumsum(KA)]
    cbB = np.r_[0, np.cumsum(KB)]
    CA, CB = int(cbA[-1]), int(cbB[-1])
    assert KA.max() <= GMAX and KB.max() <= GMAX, (KA.max(), KB.max())

    # slot arrays (per core), dummy rows are phantom rows (-1e30):
    #   window A dummy: table row 0;  window B dummy: row 25088 - BASE_B
    idxA = np.zeros((NCORES, CA * 128), np.int16)
    idxB = np.full((NCORES, CB * 128), (4 * NPCP) - BASE_B, np.int16)

    for side, idx_arr, cb, base in ((True, idxA, cbA, 0), (False, idxB, cbB, BASE_B)):
        e = np.flatnonzero(sideA == side)
        # rank within (dst) group
        e = e[np.argsort(dst[e], kind="stable")]
        de = dst[e]
        starts = np.r_[0, np.flatnonzero(np.diff(de)) + 1]
        runlen = np.diff(np.r_[starts, len(de)])
        rank = np.arange(len(de)) - np.repeat(starts, runlen)
        slot = (cb[blk[de]] + rank) * 128 + lane[de]
        val = sig[src[e]] - base
        assert val.min() >= 0 and val.max() < 32768, (val.min(), val.max())
        idx_arr[d_core[e], slot] = val.astype(np.int16)

    # wrap indices: [n] -> [128, n//16] int16, replicated across 8 groups of 16
    def wrap(a):
        n = a.shape[1]
        w = a.reshape(NCORES, n // 16, 16).transpose(0, 2, 1)  # [c, 16, n/16]
        return np.ascontiguousarray(
            np.broadcast_to(w[:, None, :, :], (NCORES, 8, 16, n // 16))
        ).reshape(NCORES, 128, n // 16)

    # gather groups: consecutive blocks, chunk budget GMAX per window
    groups = []
    b0 = 0
    while b0 < NB:
        nb = 1
        while (
            b0 + nb < NB
            and cbA[b0 + nb + 1] - cbA[b0] <= GMAX
            and cbB[b0 + nb + 1] - cbB[b0] <= GMAX
        ):
            nb += 1
        groups.append((b0, nb, int(cbA[b0]), int(cbA[b0 + nb] - cbA[b0]),
                       int(cbB[b0]), int(cbB[b0 + nb] - cbB[b0])))
        b0 += nb

    return dict(
        sig=sig, pos=pos, KA=KA, KB=KB, cbA=cbA, cbB=cbB, CA=CA, CB=CB,
        idxA=wrap(idxA), idxB=wrap(idxB), groups=groups,
        idxA_flat=idxA, idxB_flat=idxB,
    )


def _prep_weights(theta_w, theta_b, phi_w, phi_b):
    theta_w = np.asarray(theta_w, np.float32)
    phi_w = np.asarray(phi_w, np.float32)
    cb = (np.asarray(theta_b, np.float32) + np.asarray(phi_b, np.float32))
    wcat = np.concatenate(
        [theta_w.transpose(0, 2, 1), (phi_w - theta_w).transpose(0, 2, 1)], axis=2
    )  # [NL, 128(in), 256(out: y|v)]
    return np.ascontiguousarray(wcat), np.ascontiguousarray(cb)


# ----------------------------------------------------------------------------
# device kernel
# ----------------------------------------------------------------------------

def _build_kernel(g, repeats=1, loop_iters=0, loop_ag="copy8"):
    """loop_iters > 0: wrap the 4-layer pipeline in a dynamic For_i loop
    (for load-overhead-free HW timing). Collectives can't live in control
    flow, so the AllGather is replaced per `loop_ag`:
      "copy8": 8 local DMA copies filling the whole table (conservative)
      "real":  keep the collective (may refuse to compile)
    """
    import concourse.bacc as bacc
    import concourse.mybir as mybir
    import concourse.tile as tile
    from concourse.masks import make_identity

    KA, KB, groups = g["KA"], g["KB"], g["groups"]
    CA, CB = g["CA"], g["CB"]

    nc = bacc.Bacc("TRN2", target_bir_lowering=False, debug=False,
                   num_devices=NCORES)

    xin = nc.dram_tensor("xin", [NPCP, F], mybir.dt.float32, kind="ExternalInput")
    idxA_in = nc.dram_tensor("idxA", [128, CA * 8], mybir.dt.int16, kind="ExternalInput")
    idxB_in = nc.dram_tensor("idxB", [128, CB * 8], mybir.dt.int16, kind="ExternalInput")
    wcat_in = nc.dram_tensor("wcat", [NL, F, 2 * F], mybir.dt.float32, kind="ExternalInput")
    cb_in = nc.dram_tensor("cb", [NL, F], mybir.dt.float32, kind="ExternalInput")
    xout = nc.dram_tensor("xout", [NPCP, F], mybir.dt.float32, kind="ExternalOutput")

    fp32 = mybir.dt.float32
    bf16 = mybir.dt.bfloat16
    Alu = mybir.AluOpType
    Act = mybir.ActivationFunctionType

    with tile.TileContext(nc) as tc:
        with (
            tc.tile_pool(name="const", bufs=1) as constp,
            tc.tile_pool(name="xp", bufs=2) as xp,
            tc.tile_pool(name="x0p", bufs=1) as x0p,
            tc.tile_pool(name="vp", bufs=2) as vp,
            tc.tile_pool(name="wp", bufs=2) as wp,
            tc.tile_pool(name="yp", bufs=3) as yp,
            tc.tile_pool(name="xtp", bufs=3) as xtp,
            tc.tile_pool(name="ga", bufs=2 if loop_iters else 4) as gap,
            tc.tile_pool(name="gb", bufs=2 if loop_iters else 4) as gbp,
            tc.tile_pool(name="tp", bufs=8) as tp,
            tc.tile_pool(name="ps", bufs=4, space="PSUM") as ps,
            tc.tile_pool(name="dram", bufs=2, space="DRAM") as dram,
        ):
            ident = constp.tile([128, 128], fp32)
            make_identity(nc, ident[:])
            idxA = constp.tile([128, CA * 8], mybir.dt.int16)
            idxB = constp.tile([128, CB * 8], mybir.dt.int16)
            nc.sync.dma_start(idxA[:], idxA_in[:])
            nc.sync.dma_start(idxB[:], idxB_in[:])

            if loop_iters:
                x_init = x0p.tile([128, NB, F], fp32, tag="x0")
            else:
                x_init = xp.tile([128, NB, F], fp32, tag="x")
            nc.sync.dma_start(x_init[:], xin.rearrange("(b p) f -> p b f", p=128))
            x = x_init

            import contextlib
            loop_cm = tc.For_i(0, loop_iters, 1) if loop_iters else contextlib.nullcontext()
            with loop_cm:
              for l in range(NL * repeats):
                  if loop_iters and l % NL == 0:
                      x = x_init
                  li = l
                  l = l % NL
                  import contextlib as _cl
                  scope = (nc.named_scope if not loop_iters else
                           (lambda *a, **k: _cl.nullcontext()))
                  W = wp.tile([128, 2 * F], fp32, tag="w")
                  nc.sync.dma_start(W[:], wcat_in[l])
                  cb_sb = wp.tile([1, F], fp32, tag="cb")
                  nc.sync.dma_start(cb_sb[:], cb_in[l : l + 1, :])
                  cbbc = wp.tile([128, F], fp32, tag="cbbc")
                  nc.gpsimd.partition_broadcast(cbbc[:], cb_sb[:])

                  y_ag_in = dram.tile([NPCP, F], bf16, tag="yag")
                  y_all = dram.tile([NTAB, F], bf16, tag="yall",
                                    addr_space="Local" if loop_iters else "Shared")

                  v = vp.tile([128, NB, F], fp32, tag="v")

                  # ---- matmul phase: y (table) and v ----
                  mm_cm = scope(f"mm{li}")
                  mm_cm.__enter__()
                  for t in range(NB):
                      if "mm" in SKIP:
                          y_sb = yp.tile([128, F], bf16, tag="y")
                          nc.vector.memset(y_sb[:], 0.0)
                          nc.sync.dma_start(y_ag_in[t * 128 : (t + 1) * 128, :], y_sb[:])
                          nc.vector.memset(v[:, t, :], 0.0)
                          continue
                      xT_ps = ps.tile([128, 128], fp32, tag="xt_ps")
                      nc.tensor.transpose(xT_ps[:], x[:, t, :], ident[:])
                      xT = xtp.tile([128, 128], fp32, tag="xt")
                      nc.scalar.activation(xT[:], xT_ps[:], Act.Copy)
                      yv_ps = ps.tile([128, 2 * F], fp32, tag="yv_ps")
                      nc.tensor.matmul(yv_ps[:], lhsT=xT[:], rhs=W[:],
                                       start=True, stop=True)
                      y_sb = yp.tile([128, F], bf16, tag="y")
                      nc.scalar.activation(y_sb[:], yv_ps[:, 0:F], Act.Copy)
                      if t == 0:
                          nc.vector.memset(y_sb[0:N_PHANTOM, :], NEG)
                      nc.sync.dma_start(y_ag_in[t * 128 : (t + 1) * 128, :], y_sb[:])
                      nc.vector.tensor_tensor(out=v[:, t, :], in0=yv_ps[:, F : 2 * F],
                                              in1=cbbc[:], op=Alu.add)

                  mm_cm.__exit__(None, None, None)
                  ag_cm = scope(f"ag{li}")
                  ag_cm.__enter__()
                  if loop_iters and loop_ag == "copy8":
                      for c8 in range(NCORES):
                          nc.sync.dma_start(
                              y_all[c8 * NPCP:(c8 + 1) * NPCP, :], y_ag_in[:])
                  elif "ag" not in SKIP:
                      nc.gpsimd.collective_compute(
                          "AllGather",
                          Alu.bypass,
                          replica_groups=[list(range(NCORES))],
                          ins=[y_ag_in.opt()],
                          outs=[y_all.opt()],
                      )
                  else:
                      nc.sync.dma_start(y_all[0:NPCP, :], y_ag_in[:])
                  ag_cm.__exit__(None, None, None)

                  # ---- gather + segment-max phase ----
                  gr_cm = scope(f"gr{li}")
                  gr_cm.__enter__()
                  x_next = xp.tile([128, NB, F], fp32, tag="x")
                  for (b0, nbl, aoff, acnt, boff, bcnt) in groups:
                      gA = gap.tile([128, GMAX, F], bf16, tag="ga")
                      gB = gbp.tile([128, GMAX, F], bf16, tag="gb")
                      # Q7 gather ucode scratch caps num_idxs at 1024 (8 chunks)
                      if "gather" in SKIP:
                          nc.vector.memset(gA[:], 0.0)
                          nc.vector.memset(gB[:], 0.0)
                      else:
                          for o in range(0, acnt, 8):
                              n = min(8, acnt - o)
                              nc.gpsimd.dma_gather(
                                  gA[:, o : o + n, :], y_all[:, :],
                                  idxA[:, (aoff + o) * 8 : (aoff + o + n) * 8],
                                  n * 128, n * 128, F,
                              )
                          for o in range(0, bcnt, 8):
                              n = min(8, bcnt - o)
                              nc.gpsimd.dma_gather(
                                  gB[:, o : o + n, :], y_all[BASE_B:, :],
                                  idxB[:, (boff + o) * 8 : (boff + o + n) * 8],
                                  n * 128, n * 128, F,
                              )
                      ka = 0
                      kb = 0
                      for b in range(b0, b0 + nbl):
                          ha, hb = int(KA[b]), int(KB[b])
                          tS = tp.tile([128, F], fp32, tag="ts")
                          if ha > 0 and hb > 0:
                              tA = tp.tile([128, F], bf16, tag="ta")
                              tB = tp.tile([128, F], bf16, tag="tb")
                              nc.vector.tensor_reduce(
                                  out=tA[:],
                                  in_=gA[:, ka : ka + ha, :].rearrange("p c f -> p f c"),
                                  axis=mybir.AxisListType.X, op=Alu.max)
                              nc.vector.tensor_reduce(
                                  out=tB[:],
                                  in_=gB[:, kb : kb + hb, :].rearrange("p c f -> p f c"),
                                  axis=mybir.AxisListType.X, op=Alu.max)
                              tM = tp.tile([128, F], bf16, tag="tm")
                              nc.vector.tensor_tensor(out=tM[:], in0=tA[:], in1=tB[:],
                                                      op=Alu.max)
                              nc.vector.tensor_tensor(out=tS[:], in0=tM[:],
                                                      in1=v[:, b, :], op=Alu.add)
                          elif ha > 0 or hb > 0:
                              tA = tp.tile([128, F], bf16, tag="ta")
                              src_g = (gA, ka, ha) if ha > 0 else (gB, kb, hb)
                              nc.vector.tensor_reduce(
                                  out=tA[:],
                                  in_=src_g[0][:, src_g[1] : src_g[1] + src_g[2], :]
                                      .rearrange("p c f -> p f c"),
                                  axis=mybir.AxisListType.X, op=Alu.max)
                              nc.vector.tensor_tensor(out=tS[:], in0=tA[:],
                                                      in1=v[:, b, :], op=Alu.add)
                          else:
                              nc.vector.memset(tS[:], NEG)
                          # relu on Vector, not Scalar: keeps the Scalar FIFO
                          # free of reduce-phase ops so layer l+1's mm-phase
                          # PSUM copies aren't queued behind this layer's 49
                          # relus (which only drain at gather-phase end) —
                          # lets the next mm overlap this layer's gather tail
                          nc.vector.tensor_scalar(out=x_next[:, b, :], in0=tS[:],
                                                  scalar1=0.0, scalar2=None,
                                                  op0=Alu.max)
                          ka += ha
                          kb += hb
                  gr_cm.__exit__(None, None, None)
                  x = x_next

            nc.sync.dma_start(xout.rearrange("(b p) f -> p b f", p=128),
                              (x_init if loop_iters else x)[:])

    nc.compile()
    return nc


# ----------------------------------------------------------------------------
# numpy emulation of the device dataflow (for validating prep structures)
# ----------------------------------------------------------------------------

def _emulate(g, feats_dev, wcat, cb):
    KA, KB = g["KA"], g["KB"]
    x = feats_dev.copy()  # [NCORES, NPCP, F] sigma-ordered
    for l in range(NL):
        y_sh = np.einsum("cnf,fk->cnk", x, wcat[l, :, :F])
        v = np.einsum("cnf,fk->cnk", x, wcat[l, :, F:]) + cb[l]
        y_sh[:, :N_PHANTOM, :] = NEG
        table = y_sh.reshape(NTAB, F)
        xn = np.empty_like(x)
        for c in range(NCORES):
            gA = table[g["idxA_flat"][c].astype(np.int64)]          # [CA*128, F]
            gB = table[BASE_B + g["idxB_flat"][c].astype(np.int64)]
            gA = gA.reshape(g["CA"], 128, F)
            gB = gB.reshape(g["CB"], 128, F)
            for b in range(NB):
                a0, b0 = g["cbA"][b], g["cbB"][b]
                parts = []
                if KA[b] > 0:
                    parts.append(gA[a0 : a0 + KA[b]].max(0))
                if KB[b] > 0:
                    parts.append(gB[b0 : b0 + KB[b]].max(0))
                agg = np.full((128, F), NEG, np.float32) if not parts else (
                    parts[0] if len(parts) == 1 else np.maximum(*parts))
                xn[c, b * 128 : (b + 1) * 128] = np.maximum(
                    agg + v[c, b * 128 : (b + 1) * 128], 0.0)
        x = xn
    return x


def _make_in_maps(g, feats_dev, wcat, cb):
    in_maps = []
    for c in range(NCORES):
        in_maps.append({
            "xin": np.ascontiguousarray(feats_dev[c]),
            "idxA": np.ascontiguousarray(g["idxA"][c]),
            "idxB": np.ascontiguousarray(g["idxB"][c]),
            "wcat": wcat,
            "cb": cb,
        })
    return in_maps


def _feats_dev(g, feats):
    feats = np.asarray(feats, np.float32)
    fd = np.zeros((NCORES, NPCP, F), np.float32)
    core = np.arange(N) // NPC
    fd[core, g["pos"]] = feats
    return fd


def _assemble(g, results):
    out_sh = np.stack([r["xout"] for r in results])  # [NCORES, NPCP, F]
    core = np.arange(N) // NPC
    return np.ascontiguousarray(out_sh[core, g["pos"]])


def run(feats, src, dst, theta_w, theta_b, phi_w, phi_b, trace=False, repeats=1,
        loop_iters=0):
    from concourse.bass_utils import run_bass_kernel_spmd

    key = (src.tobytes()[:64], dst.tobytes()[:64], len(src))
    if _cache.get("graph_key") != key:
        _cache.clear()
        _cache["graph"] = _prep_graph(src, dst)
        _cache["graph_key"] = key
    g = _cache["graph"]
    nck = ("nc", repeats, loop_iters)
    if nck not in _cache:
        _cache[nck] = _build_kernel(g, repeats=repeats, loop_iters=loop_iters)
    nc = _cache[nck]

    wcat, cb = _prep_weights(theta_w, theta_b, phi_w, phi_b)
    feats_dev = _feats_dev(g, feats)
    in_maps = _make_in_maps(g, feats_dev, wcat, cb)
    res = run_bass_kernel_spmd(nc, in_maps, core_ids=list(range(NCORES)),
                               trace=trace)
    out = _assemble(g, res.results)
    return out, res


def kernel(feats, src, dst, theta_w, theta_b, phi_w, phi_b):
    out, _ = run(feats, src, dst, theta_w, theta_b, phi_w, phi_b)
    return out



# revision 12
# speedup vs baseline: 1.3609x; 1.3609x over previous
"""EdgeConv GNN (4 layers) on 8 Trainium2 NeuronCores.

Algebraic restructure: with y = x @ theta_w.T and
v = x @ (phi_w - theta_w).T + (phi_b + theta_b),
    msg_e = theta(x[src]-x[dst]) + theta_b + phi(x[dst]) + phi_b
          = y[src] + v[dst]
and since v[dst] is constant within a dst segment:
    out = relu(v + segment_max(y[src], dst))
(nodes with no in-edges come out of segment_max at -1e30 -> relu -> 0,
matching the reference's where(isneginf, 0) + relu).

Distribution: nodes sharded by dst across 8 cores (graph parallel).
Each layer: per-core matmuls produce its y-shard (cast to bf16) ->
AllGather the full bf16 y table to every core's DRAM -> SWDGE
dma_gather of 256B bf16 y rows by src in dst-sorted slot order ->
strided reduce_max per 128-node block (bf16), + v (f32) -> relu.

Perf notes (measured on HW):
- SWDGE dma_gather is descriptor-GENERATION-bound on the Q7: ~8.6ns/idx
  regardless of dtype (f32/bf16), call size (512/1024), queue count, or
  single_packet. GpSimd is ~89% busy; it is the kernel's critical path.
- num_idxs > 1024 per call faults the gather ucode (scratch cap).
- bf16 halves AllGather/CC time (~50us/layer) and DMA bytes; rel err
  ~3.2e-3 (tolerance 2e-2).
- 4-layer total ~4.06ms; gather gen floor for this slot layout
  (99328 slots/layer) is ~3.4ms of Q7 time.

dma_gather indices are int16 (<= 32767) so the 50176-row table is
addressed through two windows: A = rows [0, 25088) (= src cores 0-3)
and B = rows [18816, 50176) (src cores 3-7), each edge assigned to one
window (core-3 srcs balance the two).  Per-core slot structure must be
identical across cores (single SPMD instruction stream), so block
degree caps K are maxima across all 8 cores.
"""

import numpy as np

N = 50000
NCORES = 8
NPC = 6250            # real nodes per core
NPCP = 6272           # padded nodes per core (49 * 128)
F = 128
NL = 4
NB = NPCP // 128      # 49 blocks per core
NTAB = NCORES * NPCP  # 50176 table rows
A_HI = 32768          # window A = rows [0, 32768)
BASE_B = NTAB - 32768 # 17408: window B = rows [17408, NTAB)
GMAX = 40             # max chunks per gather group (per window)
NEG = -1.0e30
SKIP = set()          # debug: subset of {"gather", "reduce", "ag", "mm"}
# phantom rows sit at positions [NPC, NPCP) = block 48, lanes 106..127
PH_BLK = NPC // 128           # 48
PH_LANE = NPC - PH_BLK * 128  # 106
DUMA = NPC                     # core 0 phantom row (window A dummy)
DUMB = 3 * NPCP + NPC - BASE_B  # core 3 phantom row 25066 - base (window B dummy)

_cache = {}


# ----------------------------------------------------------------------------
# host-side graph preprocessing
# ----------------------------------------------------------------------------

def _split_counts(sig, src, dst):
    """Per-dst fixed/flex in-degree counts for the wide A/B windows."""
    srow = sig[src]
    fixedA = srow < BASE_B
    fixedB = srow >= A_HI
    flex = ~fixedA & ~fixedB
    dA0 = np.bincount(dst[fixedA], minlength=N)
    dB0 = np.bincount(dst[fixedB], minlength=N)
    dfx = np.bincount(dst[flex], minlength=N)
    return fixedA, fixedB, flex, dA0, dB0, dfx


def _balance(dA0, dB0, dfx):
    kAf = np.clip((dB0 - dA0 + dfx + 1) // 2, 0, dfx)
    return dA0 + kAf, dB0 + (dfx - kAf), kAf


def _prep_graph(src, dst):
    src = np.asarray(src).astype(np.int64)
    dst = np.asarray(dst).astype(np.int64)
    deg = np.bincount(dst, minlength=N)
    r = np.arange(N)

    # node -> (core, pos) assignment: start from a global degree-desc deal
    # (equalises per-core edge counts and per-block degree profiles), then
    # iterate: recompute window-split degrees for the current layout, resort
    # within each core by (-max(dA,dB), -(dA+dB)). Keep the best iterate.
    order = np.argsort(-deg, kind="stable")
    core = np.empty(N, np.int64)
    pos = np.empty(N, np.int64)
    core[order] = r % NCORES
    pos[order] = r // NCORES
    best = None
    for _ in range(12):
        sig = core * NPCP + pos
        _, _, _, dA0, dB0, dfx = _split_counts(sig, src, dst)
        dA, dB, _ = _balance(dA0, dB0, dfx)
        blk = pos // 128
        KA = np.zeros(NB, np.int64)
        KB = np.zeros(NB, np.int64)
        np.maximum.at(KA, blk, dA)
        np.maximum.at(KB, blk, dB)
        tot = int(KA.sum() + KB.sum())
        if best is None or tot < best[0]:
            best = (tot, pos.copy())
        k1 = np.maximum(dA, dB)
        k2 = dA + dB
        pos_n = np.empty(N, np.int64)
        for c in range(NCORES):
            ids = np.flatnonzero(core == c)
            o = np.lexsort((-k2[ids], -k1[ids]))
            pos_n[ids[o]] = np.arange(NPC)
        pos = pos_n
    pos = best[1]
    sig = core * NPCP + pos  # orig node -> table row
    fixedA, fixedB, flex, dA0, dB0, dfx = _split_counts(sig, src, dst)
    dA, dB, kAf = _balance(dA0, dB0, dfx)

    # edge side: fixed by src table row; flex edges ranked within dst group
    sideA = fixedA.copy()
    fe = np.flatnonzero(flex)
    fe = fe[np.argsort(dst[fe], kind="stable")]
    dsf = dst[fe]
    starts = np.r_[0, np.flatnonzero(np.diff(dsf)) + 1]
    runlen = np.diff(np.r_[starts, len(dsf)])
    rank = np.arange(len(dsf)) - np.repeat(starts, runlen)
    sideA[fe[rank < kAf[dsf]]] = True

    d_core = core[dst]
    blk = pos // 128
    lane = pos % 128

    # global (cross-core) block degree caps
    KA = np.zeros(NB, np.int64)
    KB = np.zeros(NB, np.int64)
    np.maximum.at(KA, blk, dA)
    np.maximum.at(KB, blk, dB)
    cbA = np.r_[0, np.cumsum(KA)]
    cbB = np.r_[0, np.cumsum(KB)]
    CA, CB = int(cbA[-1]), int(cbB[-1])
    assert KA.max() <= GMAX and KB.max() <= GMAX, (KA.max(), KB.max())

    # slot arrays (per core), dummy rows are phantom rows (-1e30)
    idxA = np.full((NCORES, CA * 128), DUMA, np.int16)
    idxB = np.full((NCORES, CB * 128), DUMB, np.int16)

    for side, idx_arr, cb, base in ((True, idxA, cbA, 0), (False, idxB, cbB, BASE_B)):
        e = np.flatnonzero(sideA == side)
        # rank within (dst) group
        e = e[np.argsort(dst[e], kind="stable")]
        de = dst[e]
        starts = np.r_[0, np.flatnonzero(np.diff(de)) + 1]
        runlen = np.diff(np.r_[starts, len(de)])
        rank = np.arange(len(de)) - np.repeat(starts, runlen)
        slot = (cb[blk[de]] + rank) * 128 + lane[de]
        val = sig[src[e]] - base
        assert val.min() >= 0 and val.max() < 32768, (val.min(), val.max())
        idx_arr[d_core[e], slot] = val.astype(np.int16)

    # wrap indices: [n] -> [128, n//16] int16, replicated across 8 groups of 16
    def wrap(a):
        n = a.shape[1]
        w = a.reshape(NCORES, n // 16, 16).transpose(0, 2, 1)  # [c, 16, n/16]
        return np.ascontiguousarray(
            np.broadcast_to(w[:, None, :, :], (NCORES, 8, 16, n // 16))
        ).reshape(NCORES, 128, n // 16)

    # gather groups: consecutive blocks, chunk budget GMAX per window
    groups = []
    b0 = 0
    while b0 < NB:
        nb = 1
        while (
            b0 + nb < NB
            and cbA[b0 + nb + 1] - cbA[b0] <= GMAX
            and cbB[b0 + nb + 1] - cbB[b0] <= GMAX
        ):
            nb += 1
        groups.append((b0, nb, int(cbA[b0]), int(cbA[b0 + nb] - cbA[b0]),
                       int(cbB[b0]), int(cbB[b0 + nb] - cbB[b0])))
        b0 += nb

    return dict(
        sig=sig, pos=pos, core=core, KA=KA, KB=KB, cbA=cbA, cbB=cbB,
        CA=CA, CB=CB, idxA=wrap(idxA), idxB=wrap(idxB), groups=groups,
        idxA_flat=idxA, idxB_flat=idxB,
    )


def _prep_weights(theta_w, theta_b, phi_w, phi_b):
    theta_w = np.asarray(theta_w, np.float32)
    phi_w = np.asarray(phi_w, np.float32)
    cb = (np.asarray(theta_b, np.float32) + np.asarray(phi_b, np.float32))
    wcat = np.concatenate(
        [theta_w.transpose(0, 2, 1), (phi_w - theta_w).transpose(0, 2, 1)], axis=2
    )  # [NL, 128(in), 256(out: y|v)]
    return np.ascontiguousarray(wcat), np.ascontiguousarray(cb)


# ----------------------------------------------------------------------------
# device kernel
# ----------------------------------------------------------------------------

def _build_kernel(g, repeats=1, loop_iters=0, loop_ag="copy8"):
    """loop_iters > 0: wrap the 4-layer pipeline in a dynamic For_i loop
    (for load-overhead-free HW timing). Collectives can't live in control
    flow, so the AllGather is replaced per `loop_ag`:
      "copy8": 8 local DMA copies filling the whole table (conservative)
      "real":  keep the collective (may refuse to compile)
    """
    import concourse.bacc as bacc
    import concourse.mybir as mybir
    import concourse.tile as tile
    from concourse.masks import make_identity

    KA, KB, groups = g["KA"], g["KB"], g["groups"]
    CA, CB = g["CA"], g["CB"]

    nc = bacc.Bacc("TRN2", target_bir_lowering=False, debug=False,
                   num_devices=NCORES)

    xin = nc.dram_tensor("xin", [NPCP, F], mybir.dt.float32, kind="ExternalInput")
    idxA_in = nc.dram_tensor("idxA", [128, CA * 8], mybir.dt.int16, kind="ExternalInput")
    idxB_in = nc.dram_tensor("idxB", [128, CB * 8], mybir.dt.int16, kind="ExternalInput")
    wcat_in = nc.dram_tensor("wcat", [NL, F, 2 * F], mybir.dt.float32, kind="ExternalInput")
    cb_in = nc.dram_tensor("cb", [NL, F], mybir.dt.float32, kind="ExternalInput")
    xout = nc.dram_tensor("xout", [NPCP, F], mybir.dt.float32, kind="ExternalOutput")

    fp32 = mybir.dt.float32
    bf16 = mybir.dt.bfloat16
    Alu = mybir.AluOpType
    Act = mybir.ActivationFunctionType

    with tile.TileContext(nc) as tc:
        with (
            tc.tile_pool(name="const", bufs=1) as constp,
            tc.tile_pool(name="xp", bufs=2) as xp,
            tc.tile_pool(name="x0p", bufs=1) as x0p,
            tc.tile_pool(name="vp", bufs=2) as vp,
            tc.tile_pool(name="wp", bufs=2) as wp,
            tc.tile_pool(name="yp", bufs=3) as yp,
            tc.tile_pool(name="xtp", bufs=3) as xtp,
            tc.tile_pool(name="ga", bufs=2 if loop_iters else 4) as gap,
            tc.tile_pool(name="gb", bufs=2 if loop_iters else 4) as gbp,
            tc.tile_pool(name="tp", bufs=8) as tp,
            tc.tile_pool(name="ps", bufs=4, space="PSUM") as ps,
            tc.tile_pool(name="dram", bufs=2, space="DRAM") as dram,
        ):
            ident = constp.tile([128, 128], fp32)
            make_identity(nc, ident[:])
            idxA = constp.tile([128, CA * 8], mybir.dt.int16)
            idxB = constp.tile([128, CB * 8], mybir.dt.int16)
            nc.sync.dma_start(idxA[:], idxA_in[:])
            nc.sync.dma_start(idxB[:], idxB_in[:])
            neg_ph = constp.tile([NPCP - NPC, F], bf16)
            nc.vector.memset(neg_ph[:], NEG)

            if loop_iters:
                x_init = x0p.tile([128, NB, F], fp32, tag="x0")
            else:
                x_init = xp.tile([128, NB, F], fp32, tag="x")
            nc.sync.dma_start(x_init[:], xin.rearrange("(b p) f -> p b f", p=128))
            x = x_init

            import contextlib
            loop_cm = tc.For_i(0, loop_iters, 1) if loop_iters else contextlib.nullcontext()
            with loop_cm:
              for l in range(NL * repeats):
                  if loop_iters and l % NL == 0:
                      x = x_init
                  li = l
                  l = l % NL
                  import contextlib as _cl
                  scope = (nc.named_scope if not loop_iters else
                           (lambda *a, **k: _cl.nullcontext()))
                  W = wp.tile([128, 2 * F], fp32, tag="w")
                  nc.sync.dma_start(W[:], wcat_in[l])
                  cb_sb = wp.tile([1, F], fp32, tag="cb")
                  nc.sync.dma_start(cb_sb[:], cb_in[l : l + 1, :])
                  cbbc = wp.tile([128, F], fp32, tag="cbbc")
                  nc.gpsimd.partition_broadcast(cbbc[:], cb_sb[:])

                  y_ag_in = dram.tile([NPCP, F], bf16, tag="yag")
                  y_all = dram.tile([NTAB, F], bf16, tag="yall",
                                    addr_space="Local" if loop_iters else "Shared")

                  v = vp.tile([128, NB, F], fp32, tag="v")

                  # ---- matmul phase: y (table) and v ----
                  mm_cm = scope(f"mm{li}")
                  mm_cm.__enter__()
                  for t in range(NB):
                      if "mm" in SKIP:
                          y_sb = yp.tile([128, F], bf16, tag="y")
                          nc.vector.memset(y_sb[:], 0.0)
                          nc.sync.dma_start(y_ag_in[t * 128 : (t + 1) * 128, :], y_sb[:])
                          nc.vector.memset(v[:, t, :], 0.0)
                          continue
                      xT_ps = ps.tile([128, 128], fp32, tag="xt_ps")
                      nc.tensor.transpose(xT_ps[:], x[:, t, :], ident[:])
                      xT = xtp.tile([128, 128], fp32, tag="xt")
                      nc.scalar.activation(xT[:], xT_ps[:], Act.Copy)
                      yv_ps = ps.tile([128, 2 * F], fp32, tag="yv_ps")
                      nc.tensor.matmul(yv_ps[:], lhsT=xT[:], rhs=W[:],
                                       start=True, stop=True)
                      y_sb = yp.tile([128, F], bf16, tag="y")
                      nc.scalar.activation(y_sb[:], yv_ps[:, 0:F], Act.Copy)
                      if t == PH_BLK:
                          nc.sync.dma_start(y_ag_in[t * 128 : NPC, :],
                                            y_sb[0:PH_LANE, :])
                          nc.sync.dma_start(y_ag_in[NPC:NPCP, :], neg_ph[:])
                      else:
                          nc.sync.dma_start(y_ag_in[t * 128 : (t + 1) * 128, :],
                                            y_sb[:])
                      nc.vector.tensor_tensor(out=v[:, t, :], in0=yv_ps[:, F : 2 * F],
                                              in1=cbbc[:], op=Alu.add)

                  mm_cm.__exit__(None, None, None)
                  ag_cm = scope(f"ag{li}")
                  ag_cm.__enter__()
                  if loop_iters and loop_ag == "copy8":
                      for c8 in range(NCORES):
                          nc.sync.dma_start(
                              y_all[c8 * NPCP:(c8 + 1) * NPCP, :], y_ag_in[:])
                  elif "ag" not in SKIP:
                      nc.gpsimd.collective_compute(
                          "AllGather",
                          Alu.bypass,
                          replica_groups=[list(range(NCORES))],
                          ins=[y_ag_in.opt()],
                          outs=[y_all.opt()],
                      )
                  else:
                      nc.sync.dma_start(y_all[0:NPCP, :], y_ag_in[:])
                  ag_cm.__exit__(None, None, None)

                  # ---- gather + segment-max phase ----
                  gr_cm = scope(f"gr{li}")
                  gr_cm.__enter__()
                  x_next = xp.tile([128, NB, F], fp32, tag="x")
                  for (b0, nbl, aoff, acnt, boff, bcnt) in groups:
                      gA = gap.tile([128, GMAX, F], bf16, tag="ga")
                      gB = gbp.tile([128, GMAX, F], bf16, tag="gb")
                      # Q7 gather ucode scratch caps num_idxs at 1024 (8 chunks)
                      if "gather" in SKIP:
                          nc.vector.memset(gA[:], 0.0)
                          nc.vector.memset(gB[:], 0.0)
                      else:
                          for o in range(0, acnt, 8):
                              n = min(8, acnt - o)
                              nc.gpsimd.dma_gather(
                                  gA[:, o : o + n, :], y_all[:, :],
                                  idxA[:, (aoff + o) * 8 : (aoff + o + n) * 8],
                                  n * 128, n * 128, F,
                              )
                          for o in range(0, bcnt, 8):
                              n = min(8, bcnt - o)
                              nc.gpsimd.dma_gather(
                                  gB[:, o : o + n, :], y_all[BASE_B:, :],
                                  idxB[:, (boff + o) * 8 : (boff + o + n) * 8],
                                  n * 128, n * 128, F,
                              )
                      ka = 0
                      kb = 0
                      for b in range(b0, b0 + nbl):
                          ha, hb = int(KA[b]), int(KB[b])
                          tS = tp.tile([128, F], fp32, tag="ts")
                          if ha > 0 and hb > 0:
                              tA = tp.tile([128, F], bf16, tag="ta")
                              tB = tp.tile([128, F], bf16, tag="tb")
                              nc.vector.tensor_reduce(
                                  out=tA[:],
                                  in_=gA[:, ka : ka + ha, :].rearrange("p c f -> p f c"),
                                  axis=mybir.AxisListType.X, op=Alu.max)
                              nc.vector.tensor_reduce(
                                  out=tB[:],
                                  in_=gB[:, kb : kb + hb, :].rearrange("p c f -> p f c"),
                                  axis=mybir.AxisListType.X, op=Alu.max)
                              tM = tp.tile([128, F], bf16, tag="tm")
                              nc.vector.tensor_tensor(out=tM[:], in0=tA[:], in1=tB[:],
                                                      op=Alu.max)
                              nc.vector.tensor_tensor(out=tS[:], in0=tM[:],
                                                      in1=v[:, b, :], op=Alu.add)
                          elif ha > 0 or hb > 0:
                              tA = tp.tile([128, F], bf16, tag="ta")
                              src_g = (gA, ka, ha) if ha > 0 else (gB, kb, hb)
                              nc.vector.tensor_reduce(
                                  out=tA[:],
                                  in_=src_g[0][:, src_g[1] : src_g[1] + src_g[2], :]
                                      .rearrange("p c f -> p f c"),
                                  axis=mybir.AxisListType.X, op=Alu.max)
                              nc.vector.tensor_tensor(out=tS[:], in0=tA[:],
                                                      in1=v[:, b, :], op=Alu.add)
                          else:
                              nc.vector.memset(tS[:], NEG)
                          # relu on Vector, not Scalar: keeps the Scalar FIFO
                          # free of reduce-phase ops so layer l+1's mm-phase
                          # PSUM copies aren't queued behind this layer's 49
                          # relus (which only drain at gather-phase end) —
                          # lets the next mm overlap this layer's gather tail
                          nc.vector.tensor_scalar(out=x_next[:, b, :], in0=tS[:],
                                                  scalar1=0.0, scalar2=None,
                                                  op0=Alu.max)
                          ka += ha
                          kb += hb
                  gr_cm.__exit__(None, None, None)
                  x = x_next

            nc.sync.dma_start(xout.rearrange("(b p) f -> p b f", p=128),
                              (x_init if loop_iters else x)[:])

    nc.compile()
    return nc


# ----------------------------------------------------------------------------
# numpy emulation of the device dataflow (for validating prep structures)
# ----------------------------------------------------------------------------

def _emulate(g, feats_dev, wcat, cb):
    KA, KB = g["KA"], g["KB"]
    x = feats_dev.copy()  # [NCORES, NPCP, F] sigma-ordered
    for l in range(NL):
        y_sh = np.einsum("cnf,fk->cnk", x, wcat[l, :, :F])
        v = np.einsum("cnf,fk->cnk", x, wcat[l, :, F:]) + cb[l]
        y_sh[:, NPC:, :] = NEG
        table = y_sh.reshape(NTAB, F)
        xn = np.empty_like(x)
        for c in range(NCORES):
            gA = table[g["idxA_flat"][c].astype(np.int64)]          # [CA*128, F]
            gB = table[BASE_B + g["idxB_flat"][c].astype(np.int64)]
            gA = gA.reshape(g["CA"], 128, F)
            gB = gB.reshape(g["CB"], 128, F)
            for b in range(NB):
                a0, b0 = g["cbA"][b], g["cbB"][b]
                parts = []
                if KA[b] > 0:
                    parts.append(gA[a0 : a0 + KA[b]].max(0))
                if KB[b] > 0:
                    parts.append(gB[b0 : b0 + KB[b]].max(0))
                agg = np.full((128, F), NEG, np.float32) if not parts else (
                    parts[0] if len(parts) == 1 else np.maximum(*parts))
                xn[c, b * 128 : (b + 1) * 128] = np.maximum(
                    agg + v[c, b * 128 : (b + 1) * 128], 0.0)
        x = xn
    return x


def _make_in_maps(g, feats_dev, wcat, cb):
    in_maps = []
    for c in range(NCORES):
        in_maps.append({
            "xin": np.ascontiguousarray(feats_dev[c]),
            "idxA": np.ascontiguousarray(g["idxA"][c]),
            "idxB": np.ascontiguousarray(g["idxB"][c]),
            "wcat": wcat,
            "cb": cb,
        })
    return in_maps


def _feats_dev(g, feats):
    feats = np.asarray(feats, np.float32)
    fd = np.zeros((NCORES, NPCP, F), np.float32)
    fd[g["core"], g["pos"]] = feats
    return fd


def _assemble(g, results):
    out_sh = np.stack([r["xout"] for r in results])  # [NCORES, NPCP, F]
    return np.ascontiguousarray(out_sh[g["core"], g["pos"]])


def run(feats, src, dst, theta_w, theta_b, phi_w, phi_b, trace=False, repeats=1,
        loop_iters=0):
    from concourse.bass_utils import run_bass_kernel_spmd

    key = (src.tobytes()[:64], dst.tobytes()[:64], len(src))
    if _cache.get("graph_key") != key:
        _cache.clear()
        _cache["graph"] = _prep_graph(src, dst)
        _cache["graph_key"] = key
    g = _cache["graph"]
    nck = ("nc", repeats, loop_iters)
    if nck not in _cache:
        _cache[nck] = _build_kernel(g, repeats=repeats, loop_iters=loop_iters)
    nc = _cache[nck]

    wcat, cb = _prep_weights(theta_w, theta_b, phi_w, phi_b)
    feats_dev = _feats_dev(g, feats)
    in_maps = _make_in_maps(g, feats_dev, wcat, cb)
    res = run_bass_kernel_spmd(nc, in_maps, core_ids=list(range(NCORES)),
                               trace=trace)
    out = _assemble(g, res.results)
    return out, res


def kernel(feats, src, dst, theta_w, theta_b, phi_w, phi_b):
    out, _ = run(feats, src, dst, theta_w, theta_b, phi_w, phi_b)
    return out

